# revision 11
# baseline (speedup 1.0000x reference)
"""Trainium2 Bass kernel for the CSA (channel self-attention) layer.

Math (per batch b, point n, channel axis c of size 128):
    q = Wq @ pos ; k = Wk @ pos ; v = Wv @ feat
    sq[n]   = sum_c q[c,n]  = (colsum Wq) . pos[:,n]
    ck[n]   = sum_c k[c,n]  = (colsum Wk) . pos[:,n]
    alpha_n = sq / (sq*ck)            (reference adds 1e-9 to the denom)
    attn    = softmax_c(alpha_n * k[:,n])
    out     = attn * v + feat

Layout strategy: per 128-point chunk we work transposed (points on
partitions, channels on the free axis) so every channel reduction is a
native free-axis op:
  - PE matmul (stationary = pos chunk) produces [kT | sq | ck] in PSUM
  - DVE tensor_tensor_reduce computes alpha*kT in-place + row max m
  - ACT computes exp(alpha*k - m) with fused row-sum s
  - DVE scalar_tensor_tensor computes uT = (expT * 1/s) * vT
  - PE transpose of uT accumulates onto PSUM preloaded with feat
    (identity matmul), so "+ feat" costs nothing extra
  - ACT evacuates the finished (attn*v + feat) chunk to SBUF

Sharding: pure data parallel, 2 batches per core on 8 cores.
"""

import os

import numpy as np

B, C, N = 16, 128, 16384
NCORES = 8
BPC = B // NCORES  # batches per core

CHUNK = 128      # points per chunk (partition dim of transposed tiles)
GROUP = 4        # chunks per PSUM group
SUPER = 2048     # points per IO supertile

# matmul dtype knob: "f32" (exact, 4 cyc/row) or "f32r" (replicated fp32,
# 1 cyc/row when the moving free dim >= 256)
MM_MODE = os.environ.get("KERNEL_MM_MODE", "f32r")

_CACHE = {}
LAST = {}


def _build(bpc: int, n: int, mm_mode: str, reps: int = 1):
    import concourse.bass as bass
    import concourse.tile as tile
    from concourse import bacc, mybir

    f32 = mybir.dt.float32
    f32r = mybir.dt.float32r
    use_r = mm_mode == "f32r"

    def mm_ap(ap):
        return ap.bitcast(f32r) if use_r else ap

    nc = bacc.Bacc()

    pos_d = nc.declare_dram_parameter("pos", [bpc, C, n], f32, isOutput=False)
    feat_d = nc.declare_dram_parameter("feat", [bpc, C, n], f32, isOutput=False)
    # wmm: [Wk.T | colsum(Wq) | colsum(Wk) | zero pad] -> (128, 256)
    wmm_d = nc.declare_dram_parameter("wmm", [C, 256], f32, isOutput=False)
    wvt_d = nc.declare_dram_parameter("wvt", [C, C], f32, isOutput=False)
    id_d = nc.declare_dram_parameter("ident", [C, C], f32, isOutput=False)
    out_d = nc.declare_dram_parameter("out", [bpc, C, n], f32, isOutput=True)

    sup = min(SUPER, n)
    n_super = n // sup
    n_groups = sup // (GROUP * CHUNK)
    mm1_cols = 256 if use_r else 130

    from contextlib import ExitStack

    with tile.TileContext(nc) as tc, ExitStack() as ctx:
        consts = ctx.enter_context(tc.tile_pool(name="consts", bufs=1))
        io_p = ctx.enter_context(tc.tile_pool(name="io_p", bufs=2))
        io_f = ctx.enter_context(tc.tile_pool(name="io_f", bufs=2))
        io_o = ctx.enter_context(tc.tile_pool(name="io_o", bufs=2))
        small = ctx.enter_context(tc.tile_pool(name="small", bufs=8))
        chunks = ctx.enter_context(tc.tile_pool(name="chunks", bufs=10))
        ps_k_pool = ctx.enter_context(tc.tile_pool(name="ps_k", bufs=4, space="PSUM"))
        ps_v_pool = ctx.enter_context(tc.tile_pool(name="ps_v", bufs=2, space="PSUM"))
        ps_o_pool = ctx.enter_context(tc.tile_pool(name="ps_o", bufs=2, space="PSUM"))

        wmm_sb = consts.tile([C, 256], f32)
        nc.sync.dma_start(out=wmm_sb[:], in_=wmm_d[:, :])
        wvt_sb = consts.tile([C, C], f32)
        nc.sync.dma_start(out=wvt_sb[:], in_=wvt_d[:, :])
        id_sb = consts.tile([C, C], f32)
        nc.sync.dma_start(out=id_sb[:], in_=id_d[:, :])

        if reps > 1:
            ctx.enter_context(tc.For_i(0, reps, 1))

        for b in range(bpc):
            for st in range(n_super):
                s0 = st * sup
                p_sb = io_p.tile([C, sup], f32)
                f_sb = io_f.tile([C, sup], f32)
                o_sb = io_o.tile([C, sup], f32)
                nc.sync.dma_start(out=p_sb[:], in_=pos_d[b][:, s0 : s0 + sup])
                nc.sync.dma_start(out=f_sb[:], in_=feat_d[b][:, s0 : s0 + sup])

                for g in range(n_groups):
                    goff = g * GROUP * CHUNK
                    ps_k = [ps_k_pool.tile([C, 512], f32, name="ps_k", tag="ps_k") for _ in range(2)]
                    ps_v = ps_v_pool.tile([C, 512], f32)
                    ps_o = ps_o_pool.tile([C, 512], f32)

                    def kslot(j):
                        # (128, mm1_cols) view of chunk j's mm1 output
                        return ps_k[j // 2][:, (j % 2) * 256 : (j % 2) * 256 + mm1_cols]

                    # mm1: [kT | sq | ck] per chunk
                    for j in range(GROUP):
                        nc.tensor.matmul(
                            out=kslot(j),
                            lhsT=mm_ap(p_sb[:, goff + j * CHUNK : goff + (j + 1) * CHUNK]),
                            rhs=mm_ap(wmm_sb[:, :mm1_cols]),
                            start=True,
                            stop=True,
                        )

                    # gather sq/ck -> (128, 4, 2), then alpha = sq / (sq*ck)
                    sqck = small.tile([C, GROUP, 2], f32)
                    for h in range(2):
                        src = ps_k[h].rearrange("p (c w) -> p c w", c=2)[:, :, 128:130]
                        nc.vector.tensor_copy(out=sqck[:, 2 * h : 2 * h + 2, :], in_=src)
                    dt_ = small.tile([C, GROUP], f32)
                    rd = small.tile([C, GROUP], f32)
                    alpha = small.tile([C, GROUP], f32)
                    nc.vector.tensor_mul(dt_, sqck[:, :, 0], sqck[:, :, 1])
                    nc.vector.reciprocal(rd, dt_)
                    nc.vector.tensor_mul(alpha, sqck[:, :, 0], rd)

                    # row max/min of kT, then m = max(alpha*mx, alpha*mn)
                    # (= rowmax of alpha*kT for either sign of alpha)
                    mxt = small.tile([C, GROUP], f32)
                    mnt = small.tile([C, GROUP], f32)
                    for j in range(GROUP):
                        kT = kslot(j)[:, :128]
                        nc.vector.reduce_max(
                            mxt[:, j : j + 1], kT, axis=mybir.AxisListType.X
                        )
                        nc.vector.tensor_reduce(
                            mnt[:, j : j + 1], kT, axis=mybir.AxisListType.X,
                            op=mybir.AluOpType.min,
                        )
                    t1 = small.tile([C, GROUP], f32)
                    t2 = small.tile([C, GROUP], f32)
                    negm = small.tile([C, GROUP], f32)
                    nc.vector.tensor_mul(t1, mxt, alpha)
                    nc.vector.tensor_mul(t2, mnt, alpha)
                    nc.vector.tensor_max(t1, t1, t2)
                    nc.vector.tensor_scalar_mul(negm, t1, -1.0)

                    # exp(alpha*kT - m) with fused row-sum; ACT applies the
                    # alpha scaling via its per-partition scale operand
                    st_s = small.tile([C, GROUP], f32)
                    expT = [chunks.tile([C, CHUNK], f32, name="expT", tag="expT") for _ in range(GROUP)]
                    for j in range(GROUP):
                        nc.scalar.activation(
                            out=expT[j],
                            in_=kslot(j)[:, :128],
                            func=mybir.ActivationFunctionType.Exp,
                            bias=negm[:, j : j + 1],
                            scale=alpha[:, j : j + 1],
                            accum_out=st_s[:, j : j + 1],
                        )
                    r_t = small.tile([C, GROUP], f32)
                    nc.vector.reciprocal(r_t, st_s)

                    # vT per chunk: stationary feat chunk, moving Wv.T
                    for j in range(GROUP):
                        nc.tensor.matmul(
                            out=ps_v[:, j * CHUNK : (j + 1) * CHUNK],
                            lhsT=mm_ap(f_sb[:, goff + j * CHUNK : goff + (j + 1) * CHUNK]),
                            rhs=mm_ap(wvt_sb),
                            start=True,
                            stop=True,
                        )

                    # preload feat into ps_o (identity matmul), then
                    # accumulate transposed uT chunks on top
                    nc.tensor.matmul(
                        out=ps_o,
                        lhsT=mm_ap(id_sb),
                        rhs=mm_ap(f_sb[:, goff : goff + GROUP * CHUNK]),
                        start=True,
                        stop=False,
                        skip_group_check=True,
                    )

                    for j in range(GROUP):
                        uT = chunks.tile([C, CHUNK], f32, name="uT", tag="uT")
                        nc.vector.scalar_tensor_tensor(
                            out=uT,
                            in0=expT[j],
                            scalar=r_t[:, j : j + 1],
                            in1=ps_v[:, j * CHUNK : (j + 1) * CHUNK],
                            op0=mybir.AluOpType.mult,
                            op1=mybir.AluOpType.mult,
                        )
                        nc.tensor.matmul(
                            out=ps_o[:, j * CHUNK : (j + 1) * CHUNK],
                            lhsT=uT,
                            rhs=id_sb,
                            is_transpose=True,
                            start=False,
                            stop=(j == GROUP - 1),
                            skip_group_check=True,
                        )

                    nc.scalar.copy(
                        out=o_sb[:, goff : goff + GROUP * CHUNK], in_=ps_o
                    )

                nc.sync.dma_start(out=out_d[b][:, s0 : s0 + sup], in_=o_sb[:])

    if not nc.is_finalized():
        nc.finalize()
    return nc


def _get_module(bpc: int, n: int, mm_mode: str, reps: int = 1):
    key = (bpc, n, mm_mode, reps)
    if key not in _CACHE:
        _CACHE[key] = _build(bpc, n, mm_mode, reps)
    return _CACHE[key]


def _host_inputs(position, feature, Wq, Wk, Wv):
    feature = np.ascontiguousarray(np.asarray(feature, dtype=np.float32))
    position = np.ascontiguousarray(np.asarray(position, dtype=np.float32))
    Wq = np.asarray(Wq, dtype=np.float32)
    Wk = np.asarray(Wk, dtype=np.float32)
    Wv = np.asarray(Wv, dtype=np.float32)
    wmm = np.zeros((C, 256), dtype=np.float32)
    wmm[:, :C] = Wk.T
    wmm[:, C] = Wq.sum(axis=0)
    wmm[:, C + 1] = Wk.sum(axis=0)
    wvt = np.ascontiguousarray(Wv.T)
    ident = np.eye(C, dtype=np.float32)
    return position, feature, wmm, wvt, ident


def kernel(feature, position, Wq, Wk, Wv):
    from concourse.bass_utils import run_bass_kernel_spmd

    position, feature, wmm, wvt, ident = _host_inputs(position, feature, Wq, Wk, Wv)

    nc = _get_module(BPC, N, MM_MODE)
    in_maps = [
        {
            "pos": np.ascontiguousarray(position[i * BPC : (i + 1) * BPC]),
            "feat": np.ascontiguousarray(feature[i * BPC : (i + 1) * BPC]),
            "wmm": wmm,
            "wvt": wvt,
            "ident": ident,
        }
        for i in range(NCORES)
    ]
    trace = os.environ.get("KERNEL_TRACE", "0") == "1"
    try:
        br = run_bass_kernel_spmd(nc, in_maps, list(range(NCORES)), trace=trace)
    except Exception:
        if not trace:
            raise
        br = run_bass_kernel_spmd(nc, in_maps, list(range(NCORES)), trace=False)
    LAST["exec_time_ns"] = br.exec_time_ns
    LAST["mean_exec_time_ns"] = br.mean_exec_time_ns
    out = np.concatenate([r["out"] for r in br.results], axis=0)
    return out.astype(np.float32)


# revision 13
# speedup vs baseline: 1.4742x; 1.4742x over previous
"""Trainium2 Bass kernel for the CSA (channel self-attention) layer.

Math (per batch b, point n, channel axis c of size 128):
    q = Wq @ pos ; k = Wk @ pos ; v = Wv @ feat
    sq[n]   = sum_c q[c,n]  = (colsum Wq) . pos[:,n]
    ck[n]   = sum_c k[c,n]  = (colsum Wk) . pos[:,n]
    alpha_n = sq / (sq*ck)            (reference adds 1e-9 to the denom)
    attn    = softmax_c(alpha_n * k[:,n])
    out     = attn * v + feat

Layout strategy: per 128-point chunk we work transposed (points on
partitions, channels on the free axis) so every channel reduction is a
native free-axis op:
  - PE matmul (stationary = pos chunk) produces [kT | sq | ck] in PSUM
  - DVE tensor_tensor_reduce computes alpha*kT in-place + row max m
  - ACT computes exp(alpha*k - m) with fused row-sum s
  - DVE scalar_tensor_tensor computes uT = (expT * 1/s) * vT
  - PE transpose of uT accumulates onto PSUM preloaded with feat
    (identity matmul), so "+ feat" costs nothing extra
  - ACT evacuates the finished (attn*v + feat) chunk to SBUF

Sharding: pure data parallel, 2 batches per core on 8 cores.
"""

import os

import numpy as np

B, C, N = 16, 128, 16384
NCORES = 8
BPC = B // NCORES  # batches per core

CHUNK = 128      # points per chunk (partition dim of transposed tiles)
GROUP = 4        # chunks per PSUM group
SUPER = 2048     # points per IO supertile

# matmul dtype knob: "f32" (exact, 4 cyc/row) or "f32r" (replicated fp32,
# 1 cyc/row when the moving free dim >= 256)
MM_MODE = os.environ.get("KERNEL_MM_MODE", "f32")
# engine placement knobs
STT_ENGINE = os.environ.get("KERNEL_STT", "dve")        # dve | gpsimd
SMALLS_ENGINE = os.environ.get("KERNEL_SMALLS", "dve")  # dve | gpsimd

_CACHE = {}
LAST = {}


def _build(bpc: int, n: int, mm_mode: str, reps: int = 1, stt_eng: str | None = None, smalls_eng: str | None = None):
    import concourse.bass as bass
    import concourse.tile as tile
    from concourse import bacc, mybir

    stt_eng = STT_ENGINE if stt_eng is None else stt_eng
    smalls_eng = SMALLS_ENGINE if smalls_eng is None else smalls_eng
    f32 = mybir.dt.float32
    f32r = mybir.dt.float32r
    use_r = mm_mode == "f32r"

    def mm_ap(ap):
        return ap.bitcast(f32r) if use_r else ap

    nc = bacc.Bacc()

    pos_d = nc.declare_dram_parameter("pos", [bpc, C, n], f32, isOutput=False)
    feat_d = nc.declare_dram_parameter("feat", [bpc, C, n], f32, isOutput=False)
    # wmm: [Wk.T | colsum(Wq) | colsum(Wk) | zero pad] -> (128, 256)
    wmm_d = nc.declare_dram_parameter("wmm", [C, 256], f32, isOutput=False)
    wvt_d = nc.declare_dram_parameter("wvt", [C, C], f32, isOutput=False)
    id_d = nc.declare_dram_parameter("ident", [C, C], f32, isOutput=False)
    out_d = nc.declare_dram_parameter("out", [bpc, C, n], f32, isOutput=True)

    sup = min(SUPER, n)
    n_super = n // sup
    n_groups = sup // (GROUP * CHUNK)
    mm1_cols = 256 if use_r else 129

    from contextlib import ExitStack

    with tile.TileContext(nc) as tc, ExitStack() as ctx:
        consts = ctx.enter_context(tc.tile_pool(name="consts", bufs=1))
        io_p = ctx.enter_context(tc.tile_pool(name="io_p", bufs=2))
        io_f = ctx.enter_context(tc.tile_pool(name="io_f", bufs=2))
        io_o = ctx.enter_context(tc.tile_pool(name="io_o", bufs=2))
        small = ctx.enter_context(tc.tile_pool(name="small", bufs=8))
        chunks = ctx.enter_context(tc.tile_pool(name="chunks", bufs=10))
        ps_k_pool = ctx.enter_context(tc.tile_pool(name="ps_k", bufs=2, space="PSUM"))
        ps_v_pool = ctx.enter_context(tc.tile_pool(name="ps_v", bufs=2, space="PSUM"))
        ps_o_pool = ctx.enter_context(tc.tile_pool(name="ps_o", bufs=2, space="PSUM"))

        wmm_sb = consts.tile([C, 256], f32)
        nc.sync.dma_start(out=wmm_sb[:], in_=wmm_d[:, :])
        wvt_sb = consts.tile([C, C], f32)
        nc.sync.dma_start(out=wvt_sb[:], in_=wvt_d[:, :])
        id_sb = consts.tile([C, C], f32)
        nc.sync.dma_start(out=id_sb[:], in_=id_d[:, :])

        if reps > 1:
            ctx.enter_context(tc.For_i(0, reps, 1))

        for b in range(bpc):
            for st in range(n_super):
                s0 = st * sup
                p_sb = io_p.tile([C, sup], f32)
                f_sb = io_f.tile([C, sup], f32)
                o_sb = io_o.tile([C, sup], f32)
                nc.sync.dma_start(out=p_sb[:], in_=pos_d[b][:, s0 : s0 + sup])
                nc.sync.dma_start(out=f_sb[:], in_=feat_d[b][:, s0 : s0 + sup])

                for g in range(n_groups):
                    goff = g * GROUP * CHUNK
                    ps_k = ps_k_pool.tile([C, GROUP * 256], f32, name="ps_k")
                    ps_v = ps_v_pool.tile([C, 512], f32)
                    ps_o = ps_o_pool.tile([C, 512], f32)
                    ps_k3 = ps_k.rearrange("p (g w) -> p g w", g=GROUP)

                    # mm1: [kT | ck] per chunk (one 256-col slot per chunk)
                    for j in range(GROUP):
                        nc.tensor.matmul(
                            out=ps_k[:, j * 256 : j * 256 + mm1_cols],
                            lhsT=mm_ap(p_sb[:, goff + j * CHUNK : goff + (j + 1) * CHUNK]),
                            rhs=mm_ap(wmm_sb[:, :mm1_cols]),
                            start=True,
                            stop=True,
                        )

                    # alpha = 1/ck  (energy_norm = sq*k/(sq*ck) = k/ck)
                    ckt = small.tile([C, GROUP, 1], f32)
                    nc.vector.tensor_copy(out=ckt, in_=ps_k3[:, :, 128:129])
                    alpha = small.tile([C, GROUP], f32)
                    nc.vector.reciprocal(alpha, ckt[:, :, 0])

                    # negated row max/min of kT (batched over the group), then
                    # negm = min(alpha*(-mx), alpha*(-mn)) = -max(alpha*mx, alpha*mn)
                    nmx = small.tile([C, GROUP], f32)
                    nmn = small.tile([C, GROUP], f32)
                    nc.vector.tensor_reduce(
                        nmx, ps_k3[:, :, 0:128], axis=mybir.AxisListType.X,
                        op=mybir.AluOpType.max, negate=True,
                    )
                    nc.vector.tensor_reduce(
                        nmn, ps_k3[:, :, 0:128], axis=mybir.AxisListType.X,
                        op=mybir.AluOpType.min, negate=True,
                    )
                    s_eng = nc.gpsimd if smalls_eng == "gpsimd" else nc.vector
                    t1 = small.tile([C, GROUP], f32)
                    t2 = small.tile([C, GROUP], f32)
                    negm = small.tile([C, GROUP], f32)
                    s_eng.tensor_mul(t1, nmx, alpha)
                    s_eng.tensor_mul(t2, nmn, alpha)
                    s_eng.tensor_tensor(negm, t1, t2, op=mybir.AluOpType.min)

                    # exp(alpha*kT - m) with fused row-sum; ACT applies the
                    # alpha scaling via its per-partition scale operand
                    st_s = small.tile([C, GROUP], f32)
                    expT = [chunks.tile([C, CHUNK], f32, name="expT", tag="expT") for _ in range(GROUP)]
                    for j in range(GROUP):
                        nc.scalar.activation(
                            out=expT[j],
                            in_=ps_k3[:, j, 0:128],
                            func=mybir.ActivationFunctionType.Exp,
                            bias=negm[:, j : j + 1],
                            scale=alpha[:, j : j + 1],
                            accum_out=st_s[:, j : j + 1],
                        )
                    r_t = small.tile([C, GROUP], f32)
                    nc.vector.reciprocal(r_t, st_s)

                    # vT per chunk: stationary feat chunk, moving Wv.T
                    for j in range(GROUP):
                        nc.tensor.matmul(
                            out=ps_v[:, j * CHUNK : (j + 1) * CHUNK],
                            lhsT=mm_ap(f_sb[:, goff + j * CHUNK : goff + (j + 1) * CHUNK]),
                            rhs=mm_ap(wvt_sb),
                            start=True,
                            stop=True,
                        )

                    # preload feat into ps_o (identity matmul), then
                    # accumulate transposed uT chunks on top
                    nc.tensor.matmul(
                        out=ps_o,
                        lhsT=mm_ap(id_sb),
                        rhs=mm_ap(f_sb[:, goff : goff + GROUP * CHUNK]),
                        start=True,
                        stop=False,
                        skip_group_check=True,
                    )

                    for j in range(GROUP):
                        uT = chunks.tile([C, CHUNK], f32, name="uT", tag="uT")
                        if stt_eng == "gpsimd":
                            vs = chunks.tile([C, CHUNK], f32, name="vs", tag="vs")
                            nc.scalar.mul(vs, ps_v[:, j * CHUNK : (j + 1) * CHUNK], r_t[:, j : j + 1])
                            nc.gpsimd.tensor_mul(uT, expT[j], vs)
                        else:
                            nc.vector.scalar_tensor_tensor(
                                out=uT,
                                in0=expT[j],
                                scalar=r_t[:, j : j + 1],
                                in1=ps_v[:, j * CHUNK : (j + 1) * CHUNK],
                                op0=mybir.AluOpType.mult,
                                op1=mybir.AluOpType.mult,
                            )
                        nc.tensor.matmul(
                            out=ps_o[:, j * CHUNK : (j + 1) * CHUNK],
                            lhsT=uT,
                            rhs=id_sb,
                            is_transpose=True,
                            start=False,
                            stop=(j == GROUP - 1),
                            skip_group_check=True,
                        )

                    nc.scalar.copy(
                        out=o_sb[:, goff : goff + GROUP * CHUNK], in_=ps_o
                    )

                nc.sync.dma_start(out=out_d[b][:, s0 : s0 + sup], in_=o_sb[:])

    if not nc.is_finalized():
        nc.finalize()
    return nc


def _get_module(bpc: int, n: int, mm_mode: str, reps: int = 1):
    key = (bpc, n, mm_mode, reps)
    if key not in _CACHE:
        _CACHE[key] = _build(bpc, n, mm_mode, reps)
    return _CACHE[key]


def _host_inputs(position, feature, Wq, Wk, Wv):
    feature = np.ascontiguousarray(np.asarray(feature, dtype=np.float32))
    position = np.ascontiguousarray(np.asarray(position, dtype=np.float32))
    Wq = np.asarray(Wq, dtype=np.float32)
    Wk = np.asarray(Wk, dtype=np.float32)
    Wv = np.asarray(Wv, dtype=np.float32)
    wmm = np.zeros((C, 256), dtype=np.float32)
    wmm[:, :C] = Wk.T
    wmm[:, C] = Wk.sum(axis=0)
    wvt = np.ascontiguousarray(Wv.T)
    ident = np.eye(C, dtype=np.float32)
    return position, feature, wmm, wvt, ident


def kernel(feature, position, Wq, Wk, Wv):
    from concourse.bass_utils import run_bass_kernel_spmd

    position, feature, wmm, wvt, ident = _host_inputs(position, feature, Wq, Wk, Wv)

    nc = _get_module(BPC, N, MM_MODE)
    in_maps = [
        {
            "pos": np.ascontiguousarray(position[i * BPC : (i + 1) * BPC]),
            "feat": np.ascontiguousarray(feature[i * BPC : (i + 1) * BPC]),
            "wmm": wmm,
            "wvt": wvt,
            "ident": ident,
        }
        for i in range(NCORES)
    ]
    trace = os.environ.get("KERNEL_TRACE", "0") == "1"
    try:
        br = run_bass_kernel_spmd(nc, in_maps, list(range(NCORES)), trace=trace)
    except Exception:
        if not trace:
            raise
        br = run_bass_kernel_spmd(nc, in_maps, list(range(NCORES)), trace=False)
    LAST["exec_time_ns"] = br.exec_time_ns
    LAST["mean_exec_time_ns"] = br.mean_exec_time_ns
    out = np.concatenate([r["out"] for r in br.results], axis=0)
    return out.astype(np.float32)
